# revision 25
# baseline (speedup 1.0000x reference)
"""Trainium2 Bass kernel for batch-8 multi-head attention (B=8, N=1024, C=768, H=12).

Distribution: pure data parallelism — batch element i runs entirely on core i
(weights replicated, zero collectives).

Per-core layout strategy (everything [channel-on-partition, token-on-free]):
  xT[k, t]            via PE transposes of the DMA'd x tiles
  qT/kT[c, t]         = W_qkv chunk (stationary) x xT (moving)      [transposed]
  v[t, c]             = xT chunk (stationary) x W_v (moving)        [natural]
  ST[s, t]            = kT slice (stationary) x qT (moving)          (scores^T)
  expST               = ACT exp(SCALE * ST), PSUM->SBUF bf16
  y65[d|sum, t]       = [v|ones] (stationary) x expST (moving): row 64 = colsum
  yT[d, t]            = y65[0:64] * broadcast(1/colsum)
  z[t, c]             = yT slice (stationary) x W_proj (moving) + b  [natural]

All matmuls bf16 x bf16 -> fp32 PSUM, 512-wide halves (PSUM bank limit).
Head loop is software-pipelined: scores/exp of head h overlap AV of head h-1.
"""
import numpy as np

import concourse.bacc as bacc
import concourse.bass as bass
import concourse.tile as tile
import concourse.mybir as mybir
from concourse import masks
from concourse.bass_utils import run_bass_kernel_spmd

F32 = mybir.dt.float32
BF16 = mybir.dt.bfloat16

B, N, C = 8, 1024, 768
H, D = 12, 64
SCALE = float(D) ** -0.5
N_CORES = 8
KT = C // 128            # 6 contraction chunks of 128
TT = N // 128            # 8 token tiles of 128
ST = N // 128            # 8 key tiles of 128
EXP_FN = mybir.ActivationFunctionType.Exp


def build_nc():
    nc = bacc.Bacc("TRN2", target_bir_lowering=False, debug=False,
                   num_devices=N_CORES)
    x_ext = nc.dram_tensor("x", [N, C], F32, kind="ExternalInput")
    wqkv_ext = nc.dram_tensor("W_qkv", [C, 3 * C], F32, kind="ExternalInput")
    wproj_ext = nc.dram_tensor("W_proj", [C, C], F32, kind="ExternalInput")
    bproj_ext = nc.dram_tensor("b_proj", [C], F32, kind="ExternalInput")
    out_ext = nc.dram_tensor("out", [N, C], F32, kind="ExternalOutput")

    with tile.TileContext(nc) as tc:
        with (
            tc.tile_pool(name="const", bufs=1) as constp,
            tc.tile_pool(name="wq", bufs=1) as wqp,
            tc.tile_pool(name="wstage", bufs=1) as wstage,
            tc.tile_pool(name="xstage", bufs=2) as xstage,
            tc.tile_pool(name="xt", bufs=1) as xtp,
            tc.tile_pool(name="qk", bufs=1) as qkp,
            tc.tile_pool(name="vp", bufs=1) as vp,
            tc.tile_pool(name="yt", bufs=1) as ytp,
            tc.tile_pool(name="yn", bufs=1) as ynp,
            tc.tile_pool(name="exp", bufs=16) as expp,
            tc.tile_pool(name="recip", bufs=1) as recipp,
            tc.tile_pool(name="z", bufs=2) as zp,
            tc.tile_pool(name="psum", bufs=2, space="PSUM") as psum,
        ):
            # ---- constants ----
            ident = constp.tile([128, 128], BF16)
            masks.make_identity(nc, ident[:])
            ones_bf = constp.tile([128, 128], BF16)
            nc.gpsimd.memset(ones_bf[:], 1.0)
            b_sb = constp.tile([1, C], F32)
            nc.sync.dma_start(b_sb[:], bproj_ext[:].rearrange("(a c) -> a c", a=1))
            b_bf = constp.tile([1, C], BF16)
            nc.vector.tensor_copy(b_bf[:], b_sb[:])
            b_bcast = constp.tile([128, C], F32)

            # ---- persistent tensors ----
            xt_bf = xtp.tile([128, KT * N], BF16)          # xT: chunk k at cols [k*N, (k+1)*N)
            wq_bf = wqp.tile([128, KT * 3 * C], BF16)      # W_qkv chunk k at cols [k*3C, ...)
            wp_bf = wqp.tile([128, KT * C], BF16)          # W_proj chunk k at cols [k*C, ...)
            qk_bf = qkp.tile([128, 12 * N], BF16)          # qT,kT: chunk co at cols [co*N, ...)
            v65 = vp.tile([128, ST * H * 65], BF16)        # per s-tile: H blocks of [v_h(64)|1]
            y_nat = ynp.tile([128, TT * C], BF16)          # y natural: t-tile t at cols [t*C, ...)
            yt_bf = ytp.tile([128, KT * N], BF16)          # yT: chunk c at cols [c*N, ...)

            def halves(width):
                out = []
                off = 0
                while off < width:
                    w = min(512, width - off)
                    out.append((off, w))
                    off += w
                return out

            # ---- phase 1+2: load x (sync queue) and W (scalar DMA queue) ----
            with nc.named_scope("xload"):
                for t in range(TT):
                    x_f = xstage.tile([128, C], F32, tag="xf")
                    nc.sync.dma_start(x_f[:], x_ext[t * 128:(t + 1) * 128, :])
                    x_bf = xstage.tile([128, C], BF16, tag="xbf")
                    nc.gpsimd.tensor_copy(x_bf[:], x_f[:])
                    for k in range(KT):
                        tag = "tp" if k % 2 == 0 else "ys"
                        tp_ps = psum.tile([128, 128], BF16, tag=tag, bufs=2)
                        nc.tensor.transpose(tp_ps[:], x_bf[:, k * 128:(k + 1) * 128], ident[:])
                        dst = xt_bf[:, k * N + t * 128: k * N + (t + 1) * 128]
                        if k % 2 == 0:
                            nc.vector.tensor_copy(dst, tp_ps[:])
                        else:
                            nc.scalar.copy(dst, tp_ps[:])

            with nc.named_scope("wload"):
                for k in range(KT):
                    w_f = wstage.tile([128, 3 * C], F32, tag=f"wf{k % 2}")
                    nc.scalar.dma_start(w_f[:], wqkv_ext[k * 128:(k + 1) * 128, :])
                    # convert in thirds (q|k|v col groups) for finer scheduling
                    for g in range(3):
                        nc.vector.tensor_copy(
                            wq_bf[:, k * 3 * C + g * C: k * 3 * C + (g + 1) * C],
                            w_f[:, g * C:(g + 1) * C])

            with nc.named_scope("wproj_load"):
                for k in range(KT):
                    w_f = wstage.tile([128, 3 * C], F32, tag=f"wf{k % 2}")
                    nc.scalar.dma_start(w_f[:, 0:C], wproj_ext[k * 128:(k + 1) * 128, :])
                    if k % 2 == 0:
                        nc.gpsimd.tensor_copy(wp_bf[:, k * C:(k + 1) * C], w_f[:, 0:C])
                    else:
                        nc.vector.tensor_copy(wp_bf[:, k * C:(k + 1) * C], w_f[:, 0:C])

                # b broadcast to 128 partitions via bf16 matmul
                b_ps = psum.tile([128, C], F32, tag="ps")
                for off, w in halves(C):
                    nc.tensor.matmul(b_ps[:, off:off + w], ones_bf[0:1, 0:128],
                                     b_bf[0:1, off:off + w], start=True, stop=True)
                nc.scalar.copy(b_bcast[:], b_ps[:])

            # ---- phase 4: qT,kT (head-pair order) with v tiles interleaved ----
            def emit_v_tile(t):
                v_ps = psum.tile([128, C], F32, tag="ps")
                for k in range(KT):
                    lhsT = xt_bf[:, k * N + t * 128: k * N + (t + 1) * 128]
                    for off, w in halves(C):
                        nc.tensor.matmul(v_ps[:, off:off + w], lhsT,
                                         wq_bf[:, k * 3 * C + 2 * C + off: k * 3 * C + 2 * C + off + w],
                                         start=(k == 0), stop=(k == KT - 1))
                base = t * H * 65
                v_view = v65[:, base: base + H * 65].rearrange("p (h w) -> p h w", w=65)
                nc.vector.tensor_copy(v_view[:, :, 0:64],
                                      v_ps[:].rearrange("p (h d) -> p h d", d=64))
                nc.gpsimd.memset(v_view[:, :, 64:65], 1.0)

            def emit_qk_col(co):
                qk_ps = psum.tile([128, N], F32, tag="ps")
                for k in range(KT):
                    lhsT = wq_bf[:, k * 3 * C + co * 128: k * 3 * C + (co + 1) * 128]
                    for off, w in halves(N):
                        nc.tensor.matmul(qk_ps[:, off:off + w], lhsT,
                                         xt_bf[:, k * N + off: k * N + off + w],
                                         start=(k == 0), stop=(k == KT - 1))
                nc.vector.tensor_copy(qk_bf[:, co * N:(co + 1) * N], qk_ps[:])

            _qs = nc.enter_named_scope("qkv", False)
            for i in range(KT):
                emit_qk_col(i)
                emit_qk_col(KT + i)
                if i < 4:
                    emit_v_tile(2 * i)
                    emit_v_tile(2 * i + 1)
            nc.leave_named_scope("qkv", _qs[0], False)

            # ---- phase 5: attention, software-pipelined across heads ----
            e_tiles = {}

            def emit_scores_exp(h):
                po = (h % 2) * 64
                q_ap = qk_bf[po:po + 64, (h // 2) * N:(h // 2 + 1) * N]
                k_ap = qk_bf[po:po + 64, (KT + h // 2) * N:(KT + h // 2 + 1) * N]
                tiles = []
                for s in range(ST):
                    s_ps = psum.tile([128, N], F32, tag="ps")
                    for off, w in halves(N):
                        nc.tensor.matmul(s_ps[:, off:off + w],
                                         k_ap[:, s * 128:(s + 1) * 128],
                                         q_ap[:, off:off + w],
                                         start=True, stop=True)
                    e_t = expp.tile([128, N], BF16, tag="exp")
                    nc.scalar.activation(e_t[:], s_ps[:], EXP_FN, bias=0.0, scale=SCALE)
                    tiles.append(e_t)
                e_tiles[h] = tiles

            def emit_av_norm(h):
                # natural-layout AV: out[t_tile][t, d|colsum]; colsum is a
                # per-partition column -> cheap reciprocal + tensor_scalar
                tiles = e_tiles.pop(h)
                for t in range(TT):
                    y_ps = psum.tile([128, 65], F32, tag="ys", bufs=2)
                    for s in range(ST):
                        lhsT = tiles[s][:, t * 128:(t + 1) * 128]
                        rhs = v65[:, s * H * 65 + h * 65: s * H * 65 + (h + 1) * 65]
                        nc.tensor.matmul(y_ps[:, 0:65], lhsT, rhs,
                                         start=(s == 0), stop=(s == ST - 1))
                    recip = recipp.tile([128, 1], F32, tag="recip", bufs=4)
                    nc.vector.reciprocal(recip[:, 0:1], y_ps[:, 64:65])
                    dst = y_nat[:, t * C + h * 64: t * C + (h + 1) * 64]
                    nc.vector.tensor_scalar_mul(dst, y_ps[:, 0:64], recip[:, 0:1])

            def emit_ytrans(i):
                # transpose y_nat c-chunk i (heads 2i, 2i+1) into yt_bf
                for t in range(TT):
                    tp_ps = psum.tile([128, 128], BF16, tag="tp")
                    nc.tensor.transpose(tp_ps[:],
                                        y_nat[:, t * C + i * 128: t * C + (i + 1) * 128],
                                        ident[:])
                    dst = yt_bf[:, i * N + t * 128: i * N + (t + 1) * 128]
                    if t % 2 == 0:
                        nc.vector.tensor_copy(dst, tp_ps[:])
                    else:
                        nc.scalar.copy(dst, tp_ps[:])

            _as = nc.enter_named_scope("attn", False)
            emit_scores_exp(0)
            for h in range(1, H):
                emit_scores_exp(h)
                emit_av_norm(h - 1)
                if (h - 1) % 2 == 1:
                    emit_ytrans((h - 1) // 2)
            emit_av_norm(H - 1)
            emit_ytrans((H - 1) // 2)
            nc.leave_named_scope("attn", _as[0], False)

            # ---- phase 6: out = yT^T @ W_proj + b ----
            _ps_ = nc.enter_named_scope("proj", False)
            for t in range(TT):
                z_ps = psum.tile([128, C], F32, tag="ps")
                for k in range(KT):
                    lhsT = yt_bf[:, k * N + t * 128: k * N + (t + 1) * 128]
                    for off, w in halves(C):
                        nc.tensor.matmul(z_ps[:, off:off + w], lhsT,
                                         wp_bf[:, k * C + off: k * C + off + w],
                                         start=(k == 0), stop=(k == KT - 1))
                z_sb = zp.tile([128, C], F32, tag="z")
                nc.vector.tensor_add(z_sb[:], z_ps[:], b_bcast[:])
                nc.sync.dma_start(out_ext[t * 128:(t + 1) * 128, :], z_sb[:])
            nc.leave_named_scope("proj", _ps_[0], False)

    nc.finalize()
    return nc


_NC = None


def _get_nc():
    global _NC
    if _NC is None:
        _NC = build_nc()
    return _NC


def _run(x, W_qkv, W_proj, b_proj, trace=False):
    nc = _get_nc()
    W_qkv = np.ascontiguousarray(W_qkv, dtype=np.float32)
    W_proj = np.ascontiguousarray(W_proj, dtype=np.float32)
    b_proj = np.ascontiguousarray(b_proj, dtype=np.float32)
    in_maps = [
        {
            "x": np.ascontiguousarray(x[i], dtype=np.float32),
            "W_qkv": W_qkv,
            "W_proj": W_proj,
            "b_proj": b_proj,
        }
        for i in range(N_CORES)
    ]
    res = run_bass_kernel_spmd(nc, in_maps, core_ids=list(range(N_CORES)),
                               trace=trace)
    out = np.stack([res.results[i]["out"] for i in range(N_CORES)], axis=0)
    return out.astype(np.float32), res


def kernel(x, W_qkv, W_proj, b_proj):
    out, _ = _run(x, W_qkv, W_proj, b_proj, trace=False)
    return out


# revision 27
# speedup vs baseline: 1.0110x; 1.0110x over previous
"""Trainium2 Bass kernel for batch-8 multi-head attention (B=8, N=1024, C=768, H=12).

Distribution: pure data parallelism — batch element i runs entirely on core i
(weights replicated, zero collectives).

Per-core layout strategy (everything [channel-on-partition, token-on-free]):
  xT[k, t]            via PE transposes of the DMA'd x tiles
  qT/kT[c, t]         = W_qkv chunk (stationary) x xT (moving)      [transposed]
  v[t, c]             = xT chunk (stationary) x W_v (moving)        [natural]
  ST[s, t]            = kT slice (stationary) x qT (moving)          (scores^T)
  expST               = ACT exp(SCALE * ST), PSUM->SBUF bf16
  y65[d|sum, t]       = [v|ones] (stationary) x expST (moving): row 64 = colsum
  yT[d, t]            = y65[0:64] * broadcast(1/colsum)
  z[t, c]             = yT slice (stationary) x W_proj (moving) + b  [natural]

All matmuls bf16 x bf16 -> fp32 PSUM, 512-wide halves (PSUM bank limit).
Head loop is software-pipelined: scores/exp of head h overlap AV of head h-1.
"""
import numpy as np

import concourse.bacc as bacc
import concourse.bass as bass
import concourse.tile as tile
import concourse.mybir as mybir
from concourse import masks
from concourse.bass_utils import run_bass_kernel_spmd

F32 = mybir.dt.float32
BF16 = mybir.dt.bfloat16

B, N, C = 8, 1024, 768
H, D = 12, 64
SCALE = float(D) ** -0.5
N_CORES = 8
KT = C // 128            # 6 contraction chunks of 128
TT = N // 128            # 8 token tiles of 128
ST = N // 128            # 8 key tiles of 128
EXP_FN = mybir.ActivationFunctionType.Exp


def build_nc():
    nc = bacc.Bacc("TRN2", target_bir_lowering=False, debug=False,
                   num_devices=N_CORES)
    x_ext = nc.dram_tensor("x", [N, C], F32, kind="ExternalInput")
    wqkv_ext = nc.dram_tensor("W_qkv", [C, 3 * C], F32, kind="ExternalInput")
    wproj_ext = nc.dram_tensor("W_proj", [C, C], F32, kind="ExternalInput")
    bproj_ext = nc.dram_tensor("b_proj", [C], F32, kind="ExternalInput")
    out_ext = nc.dram_tensor("out", [N, C], F32, kind="ExternalOutput")

    with tile.TileContext(nc) as tc:
        with (
            tc.tile_pool(name="const", bufs=1) as constp,
            tc.tile_pool(name="wq", bufs=1) as wqp,
            tc.tile_pool(name="wstage", bufs=1) as wstage,
            tc.tile_pool(name="xstage", bufs=2) as xstage,
            tc.tile_pool(name="xt", bufs=1) as xtp,
            tc.tile_pool(name="qk", bufs=1) as qkp,
            tc.tile_pool(name="vp", bufs=1) as vp,
            tc.tile_pool(name="yt", bufs=1) as ytp,
            tc.tile_pool(name="exp", bufs=16) as expp,
            tc.tile_pool(name="recip", bufs=1) as recipp,
            tc.tile_pool(name="z", bufs=2) as zp,
            tc.tile_pool(name="psum", bufs=2, space="PSUM") as psum,
        ):
            # ---- constants ----
            ident = constp.tile([128, 128], BF16)
            masks.make_identity(nc, ident[:])
            ones_bf = constp.tile([128, 128], BF16)
            nc.gpsimd.memset(ones_bf[:], 1.0)
            b_sb = constp.tile([1, C], F32)
            nc.sync.dma_start(b_sb[:], bproj_ext[:].rearrange("(a c) -> a c", a=1))
            b_bf = constp.tile([1, C], BF16)
            nc.vector.tensor_copy(b_bf[:], b_sb[:])
            b_bcast = constp.tile([128, C], F32)

            # ---- persistent tensors ----
            xt_bf = xtp.tile([128, KT * N], BF16)          # xT: chunk k at cols [k*N, (k+1)*N)
            wq_bf = wqp.tile([128, KT * 3 * C], BF16)      # W_qkv chunk k at cols [k*3C, ...)
            wp_bf = wqp.tile([128, KT * C], BF16)          # W_proj chunk k at cols [k*C, ...)
            qk_bf = qkp.tile([128, 12 * N], BF16)          # qT,kT: chunk co at cols [co*N, ...)
            v65 = vp.tile([128, ST * H * 65], BF16)        # per s-tile: H blocks of [v_h(64)|1]
            yt_bf = ytp.tile([128, KT * N], BF16)          # yT: chunk c at cols [c*N, ...)

            def halves(width):
                out = []
                off = 0
                while off < width:
                    w = min(512, width - off)
                    out.append((off, w))
                    off += w
                return out

            # ---- phase 1+2: load x (sync queue) and W (scalar DMA queue) ----
            with nc.named_scope("xload"):
                for t in range(TT):
                    x_f = xstage.tile([128, C], F32, tag="xf")
                    nc.sync.dma_start(x_f[:], x_ext[t * 128:(t + 1) * 128, :])
                    x_bf = xstage.tile([128, C], BF16, tag="xbf")
                    nc.gpsimd.tensor_copy(x_bf[:], x_f[:])
                    for k in range(KT):
                        tag = "ps" if k % 2 == 0 else "yv"
                        tp_ps = psum.tile([128, 128], BF16, tag=tag, bufs=2)
                        nc.tensor.transpose(tp_ps[:], x_bf[:, k * 128:(k + 1) * 128], ident[:])
                        dst = xt_bf[:, k * N + t * 128: k * N + (t + 1) * 128]
                        if k % 2 == 0:
                            nc.vector.tensor_copy(dst, tp_ps[:])
                        else:
                            nc.scalar.copy(dst, tp_ps[:])

            with nc.named_scope("wload"):
                for k in range(KT):
                    w_f = wstage.tile([128, 3 * C], F32, tag=f"wf{k % 2}")
                    nc.scalar.dma_start(w_f[:], wqkv_ext[k * 128:(k + 1) * 128, :])
                    # convert in thirds (q|k|v col groups) for finer scheduling
                    for g in range(3):
                        nc.vector.tensor_copy(
                            wq_bf[:, k * 3 * C + g * C: k * 3 * C + (g + 1) * C],
                            w_f[:, g * C:(g + 1) * C])

            with nc.named_scope("wproj_load"):
                for k in range(KT):
                    w_f = wstage.tile([128, 3 * C], F32, tag=f"wf{k % 2}")
                    nc.scalar.dma_start(w_f[:, 0:C], wproj_ext[k * 128:(k + 1) * 128, :])
                    if k % 2 == 0:
                        nc.gpsimd.tensor_copy(wp_bf[:, k * C:(k + 1) * C], w_f[:, 0:C])
                    else:
                        nc.vector.tensor_copy(wp_bf[:, k * C:(k + 1) * C], w_f[:, 0:C])

                # b broadcast to 128 partitions via bf16 matmul
                b_ps = psum.tile([128, C], F32, tag="ps")
                for off, w in halves(C):
                    nc.tensor.matmul(b_ps[:, off:off + w], ones_bf[0:1, 0:128],
                                     b_bf[0:1, off:off + w], start=True, stop=True)
                nc.scalar.copy(b_bcast[:], b_ps[:])

            # ---- phase 4: qT,kT (head-pair order) with v tiles interleaved ----
            def emit_v_tile(t):
                v_ps = psum.tile([128, C], F32, tag="ps")
                for k in range(KT):
                    lhsT = xt_bf[:, k * N + t * 128: k * N + (t + 1) * 128]
                    for off, w in halves(C):
                        nc.tensor.matmul(v_ps[:, off:off + w], lhsT,
                                         wq_bf[:, k * 3 * C + 2 * C + off: k * 3 * C + 2 * C + off + w],
                                         start=(k == 0), stop=(k == KT - 1))
                base = t * H * 65
                v_view = v65[:, base: base + H * 65].rearrange("p (h w) -> p h w", w=65)
                nc.vector.tensor_copy(v_view[:, :, 0:64],
                                      v_ps[:].rearrange("p (h d) -> p h d", d=64))
                nc.gpsimd.memset(v_view[:, :, 64:65], 1.0)

            def emit_qk_col(co):
                qk_ps = psum.tile([128, N], F32, tag="ps")
                for k in range(KT):
                    lhsT = wq_bf[:, k * 3 * C + co * 128: k * 3 * C + (co + 1) * 128]
                    for off, w in halves(N):
                        nc.tensor.matmul(qk_ps[:, off:off + w], lhsT,
                                         xt_bf[:, k * N + off: k * N + off + w],
                                         start=(k == 0), stop=(k == KT - 1))
                nc.vector.tensor_copy(qk_bf[:, co * N:(co + 1) * N], qk_ps[:])

            _qs = nc.enter_named_scope("qkv", False)
            for i in range(KT):
                emit_qk_col(i)
                emit_qk_col(KT + i)
                if i < 4:
                    emit_v_tile(2 * i)
                    emit_v_tile(2 * i + 1)
            nc.leave_named_scope("qkv", _qs[0], False)

            # ---- phase 5: attention, software-pipelined across heads ----
            # transposed AV (v65 stationary): y lands directly in yT layout.
            # Evacuate unnormalized y + colsum on DVE right away; reciprocals
            # batched in groups of 4 heads (bases 0/32/64/96), then per-head
            # broadcast + in-place multiply.
            cs_sb = recipp.tile([128, N], F32, tag="cs")
            recip_bf = recipp.tile([128, N], BF16, tag="recipbf")
            recip_al = recipp.tile([4, N], BF16, tag="recipal")
            nc.gpsimd.memset(cs_sb[:], 1.0)
            e_tiles = {}

            def emit_scores_exp(h):
                po = (h % 2) * 64
                q_ap = qk_bf[po:po + 64, (h // 2) * N:(h // 2 + 1) * N]
                k_ap = qk_bf[po:po + 64, (KT + h // 2) * N:(KT + h // 2 + 1) * N]
                tiles = []
                for s in range(ST):
                    s_ps = psum.tile([128, N], F32, tag="ps")
                    for off, w in halves(N):
                        nc.tensor.matmul(s_ps[:, off:off + w],
                                         k_ap[:, s * 128:(s + 1) * 128],
                                         q_ap[:, off:off + w],
                                         start=True, stop=True)
                    e_t = expp.tile([128, N], BF16, tag="exp")
                    nc.scalar.activation(e_t[:], s_ps[:], EXP_FN, bias=0.0, scale=SCALE)
                    tiles.append(e_t)
                e_tiles[h] = tiles

            def emit_av_evac(h):
                po = (h % 2) * 64
                tiles = e_tiles.pop(h)
                y_ps = psum.tile([128, N], F32, tag="yv", bufs=2)
                for s in range(ST):
                    lhsT = v65[:, s * H * 65 + h * 65: s * H * 65 + (h + 1) * 65]
                    for off, w in halves(N):
                        nc.tensor.matmul(y_ps[0:65, off:off + w], lhsT,
                                         tiles[s][:, off:off + w],
                                         start=(s == 0), stop=(s == ST - 1))
                dst = yt_bf[po:po + 64, (h // 2) * N:(h // 2 + 1) * N]
                nc.vector.tensor_copy(dst, y_ps[0:64, :])
                cb = 32 * (h % 4)
                nc.vector.tensor_copy(cs_sb[cb:cb + 1, :], y_ps[64:65, :])

            def emit_norm_group(g):
                with nc.allow_low_precision("bf16 softmax denominator"):
                    nc.vector.reciprocal(recip_bf[:, :], cs_sb[:, :])
                for h in range(4 * g, 4 * g + 4):
                    po = (h % 2) * 64
                    cb = 32 * (h % 4)
                    if cb == 0:
                        src = recip_bf
                    else:
                        nc.vector.tensor_copy(recip_al[0:1, :], recip_bf[cb:cb + 1, :])
                        src = recip_al
                    r_sb = zp.tile([128, N], BF16, tag="rbc")
                    nc.gpsimd.partition_broadcast(r_sb[:, :], src[0:1, :], channels=128)
                    dst = yt_bf[po:po + 64, (h // 2) * N:(h // 2 + 1) * N]
                    nc.vector.tensor_mul(dst, dst, r_sb[po:po + 64, :])

            _as = nc.enter_named_scope("attn", False)
            emit_scores_exp(0)
            for h in range(1, H):
                emit_av_evac(h - 1)
                emit_scores_exp(h)
                if h - 1 in (3, 7):
                    emit_norm_group((h - 1) // 4)
            emit_av_evac(H - 1)
            emit_norm_group(2)
            nc.leave_named_scope("attn", _as[0], False)

            # ---- phase 6: out = yT^T @ W_proj + b ----
            _ps_ = nc.enter_named_scope("proj", False)
            for t in range(TT):
                z_ps = psum.tile([128, C], F32, tag="ps")
                for k in range(KT):
                    lhsT = yt_bf[:, k * N + t * 128: k * N + (t + 1) * 128]
                    for off, w in halves(C):
                        nc.tensor.matmul(z_ps[:, off:off + w], lhsT,
                                         wp_bf[:, k * C + off: k * C + off + w],
                                         start=(k == 0), stop=(k == KT - 1))
                z_sb = zp.tile([128, C], F32, tag="z")
                nc.vector.tensor_add(z_sb[:], z_ps[:], b_bcast[:])
                nc.sync.dma_start(out_ext[t * 128:(t + 1) * 128, :], z_sb[:])
            nc.leave_named_scope("proj", _ps_[0], False)

    nc.finalize()
    return nc


_NC = None


def _get_nc():
    global _NC
    if _NC is None:
        _NC = build_nc()
    return _NC


def _run(x, W_qkv, W_proj, b_proj, trace=False):
    nc = _get_nc()
    W_qkv = np.ascontiguousarray(W_qkv, dtype=np.float32)
    W_proj = np.ascontiguousarray(W_proj, dtype=np.float32)
    b_proj = np.ascontiguousarray(b_proj, dtype=np.float32)
    in_maps = [
        {
            "x": np.ascontiguousarray(x[i], dtype=np.float32),
            "W_qkv": W_qkv,
            "W_proj": W_proj,
            "b_proj": b_proj,
        }
        for i in range(N_CORES)
    ]
    res = run_bass_kernel_spmd(nc, in_maps, core_ids=list(range(N_CORES)),
                               trace=trace)
    out = np.stack([res.results[i]["out"] for i in range(N_CORES)], axis=0)
    return out.astype(np.float32), res


def kernel(x, W_qkv, W_proj, b_proj):
    out, _ = _run(x, W_qkv, W_proj, b_proj, trace=False)
    return out
